# revision 13
# baseline (speedup 1.0000x reference)
"""Trainium2 Bass kernel for nn_Brown: masked directional pixel scatter + 3x3 avg.

Semantics (per image, last two dims H, W):
  pos  = prob <= 20
  avg  = 3x3 reflect-padded box mean of input
  for d in 0..7 sequentially (OFFSETS below):
      m = (dir == d) & pos
      if d == 4: x[m] = avg[m]
      else:      x[q + OFF] = input[q] for masked q (target in range),
                 then x[q] = avg[q] (for q with valid target)

Kernel formulation (validated vs reference in numpy):
  key1 = (dir+1) * (+1 if pos else -1)            in {-8..-1, 1..8}  (bf16)
  Z    = relu(key1), zeroed where the self-target is out of range    (bf16)
  out  = input copy; self-write first: out = avg where Z != 0
  for d ascending (d != 4), target rectangle p = q + OFF in range:
      u_d = relu((d+1) - Z)                  (ACT; !=0 iff Z < d+1)
      m_d = (key1[q] == d+1) * u_d           (STT; !=0 iff neighbor-write wins)
      out[p] = input[q] where m_d != 0       (copy_predicated)
  Ascending overwrite order resolves neighbor-vs-neighbor priority; the
  Z-blocking term resolves self-vs-neighbor priority exactly.

Sharding: fully data-parallel on batch, 4 batches per core x 8 cores.
"""

import numpy as np

import concourse.bass as bass
import concourse.bacc as bacc
import concourse.mybir as mybir
from concourse import tile
from concourse import bass_utils

AL = mybir.AluOpType
AF = mybir.ActivationFunctionType
DT = mybir.dt

B, C, H, W = 32, 64, 128, 128
N_CORES = 8
PB = B // N_CORES          # batches per core
NIMG = PB * C              # images per core
NGRP = NIMG // 128         # partition groups of 128 images
R = 16                     # strip rows
NSTRIP = H // R
P_THRESH = 20

# direction -> (di, dj); d=4 is the self (avg-only) case
OFFSETS = {0: (-1, -1), 1: (-1, 0), 2: (-1, 1), 3: (0, -1),
           5: (0, 1), 6: (1, -1), 7: (1, 0)}


def _register_consts(nc, values, dtype=DT.float32):
    for v in values:
        if (dtype, v) in nc.const_aps.aps:
            continue
        t = nc.alloc_sbuf_tensor(f"const-{dtype.name}-{v}", [128, 1], dtype)
        nc.gpsimd.memset(t.ap(), v)
        nc.const_aps.aps[(dtype, v)] = t.ap()
    nc.all_engine_barrier()


def build_brown(nc: bass.Bass, repeat: int = 1, variant: str = 'full'):
    """Emit the full per-core kernel into nc (one SPMD program)."""
    f32, bf16, i32 = DT.float32, DT.bfloat16, DT.int32
    _register_consts(nc, [20.5, 1.0 / 9.0] + [float(d + 1) for d in OFFSETS])
    inp = nc.dram_tensor("input", [PB, C, H, W], f32, kind="ExternalInput") \
            .ap().rearrange("b c h w -> (b c) h w")
    drm = nc.dram_tensor("dir", [PB, C, H, W], i32, kind="ExternalInput") \
            .ap().rearrange("b c h w -> (b c) h w")
    prm = nc.dram_tensor("prob", [PB, C, H, W], i32, kind="ExternalInput") \
            .ap().rearrange("b c h w -> (b c) h w")
    orm = nc.dram_tensor("out", [PB, C, H, W], f32, kind="ExternalOutput") \
            .ap().rearrange("b c h w -> (b c) h w")

    with tile.TileContext(nc) as tc:
        with tc.tile_pool(name="io", bufs=2) as pio, \
             tc.tile_pool(name="mk", bufs=2) as pmk:
            if repeat == 0:     # overhead-measurement variant: minimal work
                z = pio.tile([128, W], f32, tag="x")
                nc.sync.dma_start(z[:], inp[0:128, 0, :])
                nc.sync.dma_start(orm[0:128, 0, :], z[:])
            for _ in range(repeat):
                for g in range(NGRP):
                    for s in range(NSTRIP):
                        _strip(nc, pio, pmk, inp, drm, prm, orm, g, s, variant)
    return nc


def _strip(nc, pio, pmk, inp, drm, prm, orm, g, s, variant='full'):
    """One [128 images x R rows] strip. Tile row h <-> image row r0-1+h."""
    f32, bf16, i32 = DT.float32, DT.bfloat16, DT.int32
    r0 = s * R
    isl = slice(g * 128, (g + 1) * 128)
    first, last = (s == 0), (s == NSTRIP - 1)

    x = pio.tile([128, R + 2, W], f32, tag="x", bufs=3)
    dr = pio.tile([128, R + 2, W], i32, tag="dr", bufs=3)
    pr = pio.tile([128, R + 2, W], i32, tag="pr", bufs=3)

    # ---- loads (halo rows: reflect for input; dir/prob halo handled via key memset)
    if first:
        nc.sync.dma_start(x[:, 1:R + 2, :], inp[isl, 0:R + 1, :])
        nc.sync.dma_start(x[:, 0:1, :], inp[isl, 1:2, :])          # reflect row -1 -> 1
        nc.sync.dma_start(dr[:, 1:R + 2, :], drm[isl, 0:R + 1, :])
        nc.sync.dma_start(pr[:, 1:R + 2, :], prm[isl, 0:R + 1, :])
    elif last:
        nc.sync.dma_start(x[:, 0:R + 1, :], inp[isl, r0 - 1:H, :])
        nc.sync.dma_start(x[:, R + 1:R + 2, :], inp[isl, H - 2:H - 1, :])  # reflect
        nc.sync.dma_start(dr[:, 0:R + 1, :], drm[isl, r0 - 1:H, :])
        nc.sync.dma_start(pr[:, 0:R + 1, :], prm[isl, r0 - 1:H, :])
    else:
        nc.sync.dma_start(x[:], inp[isl, r0 - 1:r0 + R + 1, :])
        nc.sync.dma_start(dr[:], drm[isl, r0 - 1:r0 + R + 1, :])
        nc.sync.dma_start(pr[:], prm[isl, r0 - 1:r0 + R + 1, :])

    # ---- out-init copy early: only needs the x load, frees the self-CP later
    outt = pio.tile([128, R, W], f32, tag="outt", bufs=2)
    nc.sync.dma_start(outt[:], x[:, 1:R + 1, :])

    # ---- xb9 first on the ACT queue (depends only on the x load)
    xb9 = pmk.tile([128, R + 2, W], bf16, tag="xb9")
    nc.scalar.activation(xb9[:], x[:], AF.Identity, scale=1.0 / 9.0)

    # ---- key1 = (dir+1) * sign(20.5 - prob)   (bf16, R+2 rows)
    v0, v1 = (1 if first else 0), (R + 1 if last else R + 2)   # loaded row range
    vs = slice(v0, v1)
    ds1 = pmk.tile([128, R + 2, W], bf16, tag="ds1")
    ps = pmk.tile([128, R + 2, W], bf16, tag="ps")
    nc.scalar.activation(ds1[:, vs, :], dr[:, vs, :], AF.Identity, bias=1.0, scale=1.0)
    nc.scalar.activation(ps[:, vs, :], pr[:, vs, :], AF.Sign, bias=20.5, scale=-1.0)
    key = pmk.tile([128, R + 2, W], bf16, tag="key")
    nc.vector.tensor_mul(key[:, vs, :], ds1[:, vs, :], ps[:, vs, :])
    if first:
        nc.vector.memset(key[:, 0:1, :], 0.0)        # out-of-image halo: no sources
    if last:
        nc.vector.memset(key[:, R + 1:R + 2, :], 0.0)

    # ---- Z = relu(key) on the ACT engine (edge fixes come after the avg chain
    # so the vector queue has independent work while ACT produces Z)
    Z = pmk.tile([128, R, W], DT.int16, tag="Z")
    nc.scalar.activation(Z[:], key[:, 1:R + 1, :], AF.Relu)

    # ---- avg = 3x3 reflect box mean, summed in bf16 (DVE 2x mode), /9 folded
    # into the f32->bf16 convert; final upconvert to f32 on the scalar engine.
    t = pmk.tile([128, R + 2, W], bf16, tag="t")
    avgb = pmk.tile([128, R, W], bf16, tag="avgb")
    avg = pio.tile([128, R, W], f32, tag="avg", bufs=1)
    nc.vector.tensor_add(t[:, :, 1:W - 1], xb9[:, :, 0:W - 2], xb9[:, :, 2:W])
    # reflect columns on the ACT engine (vector is the bottleneck)
    nc.scalar.activation(t[:, :, 0:1], xb9[:, :, 1:2], AF.Identity, scale=2.0)
    nc.scalar.activation(t[:, :, W - 1:W], xb9[:, :, W - 2:W - 1],
                         AF.Identity, scale=2.0)
    nc.vector.tensor_add(t[:], t[:], xb9[:])
    nc.vector.tensor_add(avgb[:], t[:, 0:R, :], t[:, 2:R + 2, :])
    nc.vector.tensor_add(avgb[:], avgb[:], t[:, 1:R + 1, :])
    nc.scalar.activation(avg[:], avgb[:], AF.Identity)

    # ---- Z self-target validity fixes (Z ready by now; ACT ran during avg)
    if first:   # image row 0: self-dirs {0,1,2} (keys 1,2,3) invalid -> keep Z>=4
        nc.vector.scalar_tensor_tensor(Z[:, 0:1, :], Z[:, 0:1, :], 4.0,
                                       Z[:, 0:1, :], AL.is_ge, AL.mult)
    if last:    # image row 127: self-dirs {6,7} (keys 7,8) invalid -> keep Z<=6
        nc.vector.scalar_tensor_tensor(Z[:, R - 1:R, :], Z[:, R - 1:R, :], 6.0,
                                       Z[:, R - 1:R, :], AL.is_le, AL.mult)
    # col 0: self-dirs {0,3,6} (keys 1,4,7) invalid
    for k in (1.0, 4.0, 7.0):
        nc.vector.scalar_tensor_tensor(Z[:, :, 0:1], Z[:, :, 0:1], k,
                                       Z[:, :, 0:1], AL.not_equal, AL.mult)
    # col 127: self-dirs {2,5} (keys 3,6) invalid
    for k in (3.0, 6.0):
        nc.vector.scalar_tensor_tensor(Z[:, :, W - 1:W], Z[:, :, W - 1:W], k,
                                       Z[:, :, W - 1:W], AL.not_equal, AL.mult)

    # ---- self-write first on the out tile
    nc.vector.copy_predicated(outt[:], Z[:], avg[:])

    # ---- neighbor scan, ascending d
    for d, (di, dj) in OFFSETS.items():
        c0, c1 = max(dj, 0), W + min(dj, 0)      # target col range
        u = pmk.tile([128, R, W], bf16, tag="u")
        nc.scalar.activation(u[:], Z[:], AF.Relu, bias=float(d + 1), scale=-1.0)
        m = pmk.tile([128, R, W], DT.int16, tag="m", bufs=1)
        nc.vector.scalar_tensor_tensor(
            m[:, :, c0:c1],
            key[:, 1 - di:1 - di + R, c0 - dj:c1 - dj], float(d + 1),
            u[:, :, c0:c1], AL.is_equal, AL.mult)
        nc.vector.copy_predicated(
            outt[:, :, c0:c1], m[:, :, c0:c1],
            x[:, 1 - di:1 - di + R, c0 - dj:c1 - dj])

    nc.sync.dma_start(orm[isl, r0:r0 + R, :], outt[:])


_CACHE = {}


def _get_nc(repeat: int = 1, variant: str = "full"):
    k = ("nc", repeat, variant)
    if k not in _CACHE:
        nc = bacc.Bacc("TRN2", target_bir_lowering=False, debug=False)
        build_brown(nc, repeat=repeat, variant=variant)
        nc.compile()
        _CACHE[k] = nc
    return _CACHE[k]


def run(input, dir, prob, trace=False, trace_kwargs=None, repeat=1):
    """Shard over batch, run on 8 cores, gather. Returns (out, BassKernelResults)."""
    nc = _get_nc(repeat)
    in_maps = []
    for c in range(N_CORES):
        bs = slice(c * PB, (c + 1) * PB)
        in_maps.append({
            "input": np.ascontiguousarray(input[bs]),
            "dir": np.ascontiguousarray(dir[bs]),
            "prob": np.ascontiguousarray(prob[bs]),
        })
    res = bass_utils.run_bass_kernel_spmd(
        nc, in_maps, core_ids=list(range(N_CORES)),
        trace=trace, **(trace_kwargs or {}))
    out = np.concatenate([res.results[c]["out"] for c in range(N_CORES)], axis=0)
    return out, res


def kernel(input, dir, prob):
    input = np.asarray(input, dtype=np.float32)
    dir = np.asarray(dir, dtype=np.int32)
    prob = np.asarray(prob, dtype=np.int32)
    out, _ = run(input, dir, prob, trace=False)
    return out



# revision 14
# speedup vs baseline: 1.0029x; 1.0029x over previous
"""Trainium2 Bass kernel for nn_Brown: masked directional pixel scatter + 3x3 avg.

Semantics (per image, last two dims H, W):
  pos  = prob <= 20
  avg  = 3x3 reflect-padded box mean of input
  for d in 0..7 sequentially (OFFSETS below):
      m = (dir == d) & pos
      if d == 4: x[m] = avg[m]
      else:      x[q + OFF] = input[q] for masked q (target in range),
                 then x[q] = avg[q] (for q with valid target)

Kernel formulation (validated vs reference in numpy):
  key1 = (dir+1) * (+1 if pos else -1)            in {-8..-1, 1..8}  (bf16)
  Z    = relu(key1), zeroed where the self-target is out of range    (bf16)
  out  = input copy; self-write first: out = avg where Z != 0
  for d ascending (d != 4), target rectangle p = q + OFF in range:
      u_d = relu((d+1) - Z)                  (ACT; !=0 iff Z < d+1)
      m_d = (key1[q] == d+1) * u_d           (STT; !=0 iff neighbor-write wins)
      out[p] = input[q] where m_d != 0       (copy_predicated)
  Ascending overwrite order resolves neighbor-vs-neighbor priority; the
  Z-blocking term resolves self-vs-neighbor priority exactly.

Sharding: fully data-parallel on batch, 4 batches per core x 8 cores.
"""

import numpy as np

import concourse.bass as bass
import concourse.bacc as bacc
import concourse.mybir as mybir
from concourse import tile
from concourse import bass_utils

AL = mybir.AluOpType
AF = mybir.ActivationFunctionType
DT = mybir.dt

B, C, H, W = 32, 64, 128, 128
N_CORES = 8
PB = B // N_CORES          # batches per core
NIMG = PB * C              # images per core
NGRP = NIMG // 128         # partition groups of 128 images
R = 16                     # strip rows
NSTRIP = H // R
P_THRESH = 20

# direction -> (di, dj); d=4 is the self (avg-only) case
OFFSETS = {0: (-1, -1), 1: (-1, 0), 2: (-1, 1), 3: (0, -1),
           5: (0, 1), 6: (1, -1), 7: (1, 0)}


def _register_consts(nc, values, dtype=DT.float32):
    for v in values:
        if (dtype, v) in nc.const_aps.aps:
            continue
        t = nc.alloc_sbuf_tensor(f"const-{dtype.name}-{v}", [128, 1], dtype)
        nc.gpsimd.memset(t.ap(), v)
        nc.const_aps.aps[(dtype, v)] = t.ap()
    nc.all_engine_barrier()


def build_brown(nc: bass.Bass, repeat: int = 1, variant: str = 'full'):
    """Emit the full per-core kernel into nc (one SPMD program)."""
    f32, bf16, i32 = DT.float32, DT.bfloat16, DT.int32
    _register_consts(nc, [20.5, 1.0 / 9.0] + [float(d + 1) for d in OFFSETS])
    inp = nc.dram_tensor("input", [PB, C, H, W], f32, kind="ExternalInput") \
            .ap().rearrange("b c h w -> (b c) h w")
    drm = nc.dram_tensor("dir", [PB, C, H, W], i32, kind="ExternalInput") \
            .ap().rearrange("b c h w -> (b c) h w")
    prm = nc.dram_tensor("prob", [PB, C, H, W], i32, kind="ExternalInput") \
            .ap().rearrange("b c h w -> (b c) h w")
    orm = nc.dram_tensor("out", [PB, C, H, W], f32, kind="ExternalOutput") \
            .ap().rearrange("b c h w -> (b c) h w")

    with tile.TileContext(nc) as tc:
        with tc.tile_pool(name="io", bufs=2) as pio, \
             tc.tile_pool(name="mk", bufs=2) as pmk:
            if repeat == 0:     # overhead-measurement variant: minimal work
                z = pio.tile([128, W], f32, tag="x")
                nc.sync.dma_start(z[:], inp[0:128, 0, :])
                nc.sync.dma_start(orm[0:128, 0, :], z[:])
            for _ in range(repeat):
                for g in range(NGRP):
                    for s in range(NSTRIP):
                        _strip(nc, pio, pmk, inp, drm, prm, orm, g, s, variant)
    return nc


def _strip(nc, pio, pmk, inp, drm, prm, orm, g, s, variant='full'):
    """One [128 images x R rows] strip. Tile row h <-> image row r0-1+h."""
    f32, bf16, i32 = DT.float32, DT.bfloat16, DT.int32
    r0 = s * R
    isl = slice(g * 128, (g + 1) * 128)
    first, last = (s == 0), (s == NSTRIP - 1)

    x = pio.tile([128, R + 2, W], f32, tag="x", bufs=3)
    dr = pio.tile([128, R + 2, W], i32, tag="dr", bufs=3)
    pr = pio.tile([128, R + 2, W], i32, tag="pr", bufs=3)

    # ---- loads (halo rows: reflect for input; dir/prob halo handled via key memset)
    if first:
        nc.sync.dma_start(x[:, 1:R + 2, :], inp[isl, 0:R + 1, :])
        nc.sync.dma_start(x[:, 0:1, :], inp[isl, 1:2, :])          # reflect row -1 -> 1
        nc.sync.dma_start(dr[:, 1:R + 2, :], drm[isl, 0:R + 1, :])
        nc.sync.dma_start(pr[:, 1:R + 2, :], prm[isl, 0:R + 1, :])
    elif last:
        nc.sync.dma_start(x[:, 0:R + 1, :], inp[isl, r0 - 1:H, :])
        nc.sync.dma_start(x[:, R + 1:R + 2, :], inp[isl, H - 2:H - 1, :])  # reflect
        nc.sync.dma_start(dr[:, 0:R + 1, :], drm[isl, r0 - 1:H, :])
        nc.sync.dma_start(pr[:, 0:R + 1, :], prm[isl, r0 - 1:H, :])
    else:
        nc.sync.dma_start(x[:], inp[isl, r0 - 1:r0 + R + 1, :])
        nc.sync.dma_start(dr[:], drm[isl, r0 - 1:r0 + R + 1, :])
        nc.sync.dma_start(pr[:], prm[isl, r0 - 1:r0 + R + 1, :])

    # ---- key1 = (dir+1) * sign(20.5 - prob)   (bf16, R+2 rows)
    v0, v1 = (1 if first else 0), (R + 1 if last else R + 2)   # loaded row range
    vs = slice(v0, v1)
    ds1 = pmk.tile([128, R + 2, W], bf16, tag="ds1")
    ps = pmk.tile([128, R + 2, W], bf16, tag="ps")
    nc.scalar.activation(ds1[:, vs, :], dr[:, vs, :], AF.Identity, bias=1.0, scale=1.0)
    nc.scalar.activation(ps[:, vs, :], pr[:, vs, :], AF.Sign, bias=20.5, scale=-1.0)
    key = pmk.tile([128, R + 2, W], bf16, tag="key")
    nc.vector.tensor_mul(key[:, vs, :], ds1[:, vs, :], ps[:, vs, :])
    if first:
        nc.vector.memset(key[:, 0:1, :], 0.0)        # out-of-image halo: no sources
    if last:
        nc.vector.memset(key[:, R + 1:R + 2, :], 0.0)

    # ---- Z = relu(key) with out-of-range self-targets zeroed
    # (int16: copy_predicated masks must be integer dtype per BIR verifier)
    Z = pmk.tile([128, R, W], DT.int16, tag="Z")
    nc.scalar.activation(Z[:], key[:, 1:R + 1, :], AF.Relu)   # relu on ACT engine
    if first:   # image row 0: self-dirs {0,1,2} (keys 1,2,3) invalid -> keep Z>=4
        nc.vector.scalar_tensor_tensor(Z[:, 0:1, :], Z[:, 0:1, :], 4.0,
                                       Z[:, 0:1, :], AL.is_ge, AL.mult)
    if last:    # image row 127: self-dirs {6,7} (keys 7,8) invalid -> keep Z<=6
        nc.vector.scalar_tensor_tensor(Z[:, R - 1:R, :], Z[:, R - 1:R, :], 6.0,
                                       Z[:, R - 1:R, :], AL.is_le, AL.mult)
    # col 0: self-dirs {0,3,6} (keys 1,4,7) invalid
    for k in (1.0, 4.0, 7.0):
        nc.vector.scalar_tensor_tensor(Z[:, :, 0:1], Z[:, :, 0:1], k,
                                       Z[:, :, 0:1], AL.not_equal, AL.mult)
    # col 127: self-dirs {2,5} (keys 3,6) invalid
    for k in (3.0, 6.0):
        nc.vector.scalar_tensor_tensor(Z[:, :, W - 1:W], Z[:, :, W - 1:W], k,
                                       Z[:, :, W - 1:W], AL.not_equal, AL.mult)

    # ---- avg = 3x3 reflect box mean, summed in bf16 (DVE 2x mode), /9 folded
    # into the f32->bf16 convert; final upconvert to f32 on the scalar engine.
    do_avg = variant not in ("noavg", "min")
    do_scan = variant not in ("noscan", "min")
    xb9 = pmk.tile([128, R + 2, W], bf16, tag="xb9")
    t = pmk.tile([128, R + 2, W], bf16, tag="t")
    avgb = pmk.tile([128, R, W], bf16, tag="avgb")
    avg = pio.tile([128, R, W], f32, tag="avg", bufs=1)
    if do_avg:
        nc.scalar.activation(xb9[:], x[:], AF.Identity, scale=1.0 / 9.0)
        nc.vector.tensor_add(t[:, :, 1:W - 1], xb9[:, :, 0:W - 2], xb9[:, :, 2:W])
        # reflect columns on the ACT engine (vector is the bottleneck)
        nc.scalar.activation(t[:, :, 0:1], xb9[:, :, 1:2], AF.Identity, scale=2.0)
        nc.scalar.activation(t[:, :, W - 1:W], xb9[:, :, W - 2:W - 1],
                             AF.Identity, scale=2.0)
        nc.vector.tensor_add(t[:], t[:], xb9[:])
        nc.vector.tensor_add(avgb[:], t[:, 0:R, :], t[:, 2:R + 2, :])
        nc.vector.tensor_add(avgb[:], avgb[:], t[:, 1:R + 1, :])
        nc.scalar.activation(avg[:], avgb[:], AF.Identity)

    # ---- out = input; self-write first
    outt = pio.tile([128, R, W], f32, tag="outt", bufs=2)
    nc.sync.dma_start(outt[:], x[:, 1:R + 1, :])
    if do_avg:
        nc.vector.copy_predicated(outt[:], Z[:], avg[:])

    # ---- neighbor scan, ascending d
    for d, (di, dj) in (OFFSETS.items() if do_scan else []):
        c0, c1 = max(dj, 0), W + min(dj, 0)      # target col range
        u = pmk.tile([128, R, W], bf16, tag="u")
        nc.scalar.activation(u[:], Z[:], AF.Relu, bias=float(d + 1), scale=-1.0)
        m = pmk.tile([128, R, W], DT.int16, tag="m", bufs=1)
        nc.vector.scalar_tensor_tensor(
            m[:, :, c0:c1],
            key[:, 1 - di:1 - di + R, c0 - dj:c1 - dj], float(d + 1),
            u[:, :, c0:c1], AL.is_equal, AL.mult)
        nc.vector.copy_predicated(
            outt[:, :, c0:c1], m[:, :, c0:c1],
            x[:, 1 - di:1 - di + R, c0 - dj:c1 - dj])

    nc.sync.dma_start(orm[isl, r0:r0 + R, :], outt[:])


_CACHE = {}


def _get_nc(repeat: int = 1, variant: str = "full"):
    k = ("nc", repeat, variant)
    if k not in _CACHE:
        nc = bacc.Bacc("TRN2", target_bir_lowering=False, debug=False)
        build_brown(nc, repeat=repeat, variant=variant)
        nc.compile()
        _CACHE[k] = nc
    return _CACHE[k]


def run(input, dir, prob, trace=False, trace_kwargs=None, repeat=1):
    """Shard over batch, run on 8 cores, gather. Returns (out, BassKernelResults)."""
    nc = _get_nc(repeat)
    in_maps = []
    for c in range(N_CORES):
        bs = slice(c * PB, (c + 1) * PB)
        in_maps.append({
            "input": np.ascontiguousarray(input[bs]),
            "dir": np.ascontiguousarray(dir[bs]),
            "prob": np.ascontiguousarray(prob[bs]),
        })
    res = bass_utils.run_bass_kernel_spmd(
        nc, in_maps, core_ids=list(range(N_CORES)),
        trace=trace, **(trace_kwargs or {}))
    out = np.concatenate([res.results[c]["out"] for c in range(N_CORES)], axis=0)
    return out, res


def kernel(input, dir, prob):
    input = np.asarray(input, dtype=np.float32)
    dir = np.asarray(dir, dtype=np.int32)
    prob = np.asarray(prob, dtype=np.int32)
    out, _ = run(input, dir, prob, trace=False)
    return out



# revision 16
# speedup vs baseline: 1.0031x; 1.0002x over previous
"""Trainium2 Bass kernel for nn_Brown: masked directional pixel scatter + 3x3 avg.

Semantics (per image, last two dims H, W):
  pos  = prob <= 20
  avg  = 3x3 reflect-padded box mean of input
  for d in 0..7 sequentially (OFFSETS below):
      m = (dir == d) & pos
      if d == 4: x[m] = avg[m]
      else:      x[q + OFF] = input[q] for masked q (target in range),
                 then x[q] = avg[q] (for q with valid target)

Kernel formulation (validated vs reference in numpy):
  key1 = (dir+1) * (+1 if pos else -1)            in {-8..-1, 1..8}  (bf16)
  Z    = relu(key1), zeroed where the self-target is out of range    (bf16)
  out  = input copy; self-write first: out = avg where Z != 0
  for d ascending (d != 4), target rectangle p = q + OFF in range:
      u_d = relu((d+1) - Z)                  (ACT; !=0 iff Z < d+1)
      m_d = (key1[q] == d+1) * u_d           (STT; !=0 iff neighbor-write wins)
      out[p] = input[q] where m_d != 0       (copy_predicated)
  Ascending overwrite order resolves neighbor-vs-neighbor priority; the
  Z-blocking term resolves self-vs-neighbor priority exactly.

Sharding: fully data-parallel on batch, 4 batches per core x 8 cores.
"""

import numpy as np

import concourse.bass as bass
import concourse.bacc as bacc
import concourse.mybir as mybir
from concourse import tile
from concourse import bass_utils

AL = mybir.AluOpType
AF = mybir.ActivationFunctionType
DT = mybir.dt

B, C, H, W = 32, 64, 128, 128
N_CORES = 8
PB = B // N_CORES          # batches per core
NIMG = PB * C              # images per core
NGRP = NIMG // 128         # partition groups of 128 images
R = 16                     # strip rows
NSTRIP = H // R
P_THRESH = 20

# direction -> (di, dj); d=4 is the self (avg-only) case
OFFSETS = {0: (-1, -1), 1: (-1, 0), 2: (-1, 1), 3: (0, -1),
           5: (0, 1), 6: (1, -1), 7: (1, 0)}


def _register_consts(nc, values, dtype=DT.float32):
    for v in values:
        if (dtype, v) in nc.const_aps.aps:
            continue
        t = nc.alloc_sbuf_tensor(f"const-{dtype.name}-{v}", [128, 1], dtype)
        nc.gpsimd.memset(t.ap(), v)
        nc.const_aps.aps[(dtype, v)] = t.ap()
    nc.all_engine_barrier()


def build_brown(nc: bass.Bass, repeat: int = 1, variant: str = 'full'):
    """Emit the full per-core kernel into nc (one SPMD program)."""
    f32, bf16, i32 = DT.float32, DT.bfloat16, DT.int32
    _register_consts(nc, [20.5, 1.0 / 9.0] + [float(d + 1) for d in OFFSETS])
    inp = nc.dram_tensor("input", [PB, C, H, W], f32, kind="ExternalInput") \
            .ap().rearrange("b c h w -> (b c) h w")
    drm = nc.dram_tensor("dir", [PB, C, H, W], i32, kind="ExternalInput") \
            .ap().rearrange("b c h w -> (b c) h w")
    prm = nc.dram_tensor("prob", [PB, C, H, W], i32, kind="ExternalInput") \
            .ap().rearrange("b c h w -> (b c) h w")
    orm = nc.dram_tensor("out", [PB, C, H, W], f32, kind="ExternalOutput") \
            .ap().rearrange("b c h w -> (b c) h w")

    with tile.TileContext(nc) as tc:
        with tc.tile_pool(name="io", bufs=2) as pio, \
             tc.tile_pool(name="mk", bufs=2) as pmk:
            if repeat == 0:     # overhead-measurement variant: minimal work
                z = pio.tile([128, W], f32, tag="x")
                nc.sync.dma_start(z[:], inp[0:128, 0, :])
                nc.sync.dma_start(orm[0:128, 0, :], z[:])
            for _ in range(repeat):
                for g in range(NGRP):
                    for s in range(NSTRIP):
                        _strip(nc, pio, pmk, inp, drm, prm, orm, g, s, variant)
    return nc


def _strip(nc, pio, pmk, inp, drm, prm, orm, g, s, variant='full'):
    """One [128 images x R rows] strip. Tile row h <-> image row r0-1+h."""
    f32, bf16, i32 = DT.float32, DT.bfloat16, DT.int32
    r0 = s * R
    isl = slice(g * 128, (g + 1) * 128)
    first, last = (s == 0), (s == NSTRIP - 1)

    x = pio.tile([128, R + 2, W], f32, tag="x", bufs=3)
    dr = pio.tile([128, R + 2, W], i32, tag="dr", bufs=3)
    pr = pio.tile([128, R + 2, W], i32, tag="pr", bufs=3)

    # ---- loads (halo rows: reflect for input; dir/prob halo handled via key memset)
    if first:
        nc.sync.dma_start(x[:, 1:R + 2, :], inp[isl, 0:R + 1, :])
        nc.sync.dma_start(x[:, 0:1, :], inp[isl, 1:2, :])          # reflect row -1 -> 1
        nc.sync.dma_start(dr[:, 1:R + 2, :], drm[isl, 0:R + 1, :])
        nc.sync.dma_start(pr[:, 1:R + 2, :], prm[isl, 0:R + 1, :])
    elif last:
        nc.sync.dma_start(x[:, 0:R + 1, :], inp[isl, r0 - 1:H, :])
        nc.sync.dma_start(x[:, R + 1:R + 2, :], inp[isl, H - 2:H - 1, :])  # reflect
        nc.sync.dma_start(dr[:, 0:R + 1, :], drm[isl, r0 - 1:H, :])
        nc.sync.dma_start(pr[:, 0:R + 1, :], prm[isl, r0 - 1:H, :])
    else:
        nc.sync.dma_start(x[:], inp[isl, r0 - 1:r0 + R + 1, :])
        nc.sync.dma_start(dr[:], drm[isl, r0 - 1:r0 + R + 1, :])
        nc.sync.dma_start(pr[:], prm[isl, r0 - 1:r0 + R + 1, :])

    # ---- key1 = (dir+1) * sign(20.5 - prob)   (bf16, R+2 rows)
    v0, v1 = (1 if first else 0), (R + 1 if last else R + 2)   # loaded row range
    vs = slice(v0, v1)
    ds1 = pmk.tile([128, R + 2, W], bf16, tag="ds1")
    ps = pmk.tile([128, R + 2, W], bf16, tag="ps")
    nc.scalar.activation(ds1[:, vs, :], dr[:, vs, :], AF.Identity, bias=1.0, scale=1.0)
    nc.scalar.activation(ps[:, vs, :], pr[:, vs, :], AF.Sign, bias=20.5, scale=-1.0)
    key = pmk.tile([128, R + 2, W], bf16, tag="key")
    nc.vector.tensor_mul(key[:, vs, :], ds1[:, vs, :], ps[:, vs, :])
    if first:
        nc.vector.memset(key[:, 0:1, :], 0.0)        # out-of-image halo: no sources
    if last:
        nc.vector.memset(key[:, R + 1:R + 2, :], 0.0)

    # ---- Z = relu(key) with out-of-range self-targets zeroed
    # (int16: copy_predicated masks must be integer dtype per BIR verifier)
    Z = pmk.tile([128, R, W], DT.int16, tag="Z")
    nc.scalar.activation(Z[:], key[:, 1:R + 1, :], AF.Relu)   # relu on ACT engine
    if first:   # image row 0: self-dirs {0,1,2} (keys 1,2,3) invalid -> keep Z>=4
        nc.vector.scalar_tensor_tensor(Z[:, 0:1, :], Z[:, 0:1, :], 4.0,
                                       Z[:, 0:1, :], AL.is_ge, AL.mult)
    if last:    # image row 127: self-dirs {6,7} (keys 7,8) invalid -> keep Z<=6
        nc.vector.scalar_tensor_tensor(Z[:, R - 1:R, :], Z[:, R - 1:R, :], 6.0,
                                       Z[:, R - 1:R, :], AL.is_le, AL.mult)
    # col 0: self-dirs {0,3,6} (keys 1,4,7) invalid
    for k in (1.0, 4.0, 7.0):
        nc.vector.scalar_tensor_tensor(Z[:, :, 0:1], Z[:, :, 0:1], k,
                                       Z[:, :, 0:1], AL.not_equal, AL.mult)
    # col 127: self-dirs {2,5} (keys 3,6) invalid
    for k in (3.0, 6.0):
        nc.vector.scalar_tensor_tensor(Z[:, :, W - 1:W], Z[:, :, W - 1:W], k,
                                       Z[:, :, W - 1:W], AL.not_equal, AL.mult)

    # ---- avg = 3x3 reflect box mean, summed in bf16 (DVE 2x mode), /9 folded
    # into the f32->bf16 convert; final upconvert to f32 on the scalar engine.
    do_avg = variant not in ("noavg", "min")
    do_scan = variant not in ("noscan", "min")
    xb9 = pmk.tile([128, R + 2, W], bf16, tag="xb9")
    t = pmk.tile([128, R + 2, W], bf16, tag="t")
    avgb = pmk.tile([128, R, W], bf16, tag="avgb")
    avg = pio.tile([128, R, W], f32, tag="avg", bufs=2)
    if do_avg:
        nc.scalar.activation(xb9[:], x[:], AF.Identity, scale=1.0 / 9.0)
        nc.vector.tensor_add(t[:, :, 1:W - 1], xb9[:, :, 0:W - 2], xb9[:, :, 2:W])
        # reflect columns on the ACT engine (vector is the bottleneck)
        nc.scalar.activation(t[:, :, 0:1], xb9[:, :, 1:2], AF.Identity, scale=2.0)
        nc.scalar.activation(t[:, :, W - 1:W], xb9[:, :, W - 2:W - 1],
                             AF.Identity, scale=2.0)
        nc.vector.tensor_add(t[:], t[:], xb9[:])
        nc.vector.tensor_add(avgb[:], t[:, 0:R, :], t[:, 2:R + 2, :])
        nc.vector.tensor_add(avgb[:], avgb[:], t[:, 1:R + 1, :])
        nc.scalar.activation(avg[:], avgb[:], AF.Identity)

    # ---- out = input; self-write first
    outt = pio.tile([128, R, W], f32, tag="outt", bufs=3)
    nc.sync.dma_start(outt[:], x[:, 1:R + 1, :])
    if do_avg:
        nc.vector.copy_predicated(outt[:], Z[:], avg[:])

    # ---- neighbor scan, ascending d
    for d, (di, dj) in (OFFSETS.items() if do_scan else []):
        c0, c1 = max(dj, 0), W + min(dj, 0)      # target col range
        u = pmk.tile([128, R, W], bf16, tag="u")
        nc.scalar.activation(u[:], Z[:], AF.Relu, bias=float(d + 1), scale=-1.0)
        m = pmk.tile([128, R, W], DT.int16, tag="m", bufs=1)
        nc.vector.scalar_tensor_tensor(
            m[:, :, c0:c1],
            key[:, 1 - di:1 - di + R, c0 - dj:c1 - dj], float(d + 1),
            u[:, :, c0:c1], AL.is_equal, AL.mult)
        nc.vector.copy_predicated(
            outt[:, :, c0:c1], m[:, :, c0:c1],
            x[:, 1 - di:1 - di + R, c0 - dj:c1 - dj])

    nc.sync.dma_start(orm[isl, r0:r0 + R, :], outt[:])


_CACHE = {}


def _get_nc(repeat: int = 1, variant: str = "full"):
    k = ("nc", repeat, variant)
    if k not in _CACHE:
        nc = bacc.Bacc("TRN2", target_bir_lowering=False, debug=False)
        build_brown(nc, repeat=repeat, variant=variant)
        nc.compile()
        _CACHE[k] = nc
    return _CACHE[k]


def run(input, dir, prob, trace=False, trace_kwargs=None, repeat=1):
    """Shard over batch, run on 8 cores, gather. Returns (out, BassKernelResults)."""
    nc = _get_nc(repeat)
    in_maps = []
    for c in range(N_CORES):
        bs = slice(c * PB, (c + 1) * PB)
        in_maps.append({
            "input": np.ascontiguousarray(input[bs]),
            "dir": np.ascontiguousarray(dir[bs]),
            "prob": np.ascontiguousarray(prob[bs]),
        })
    res = bass_utils.run_bass_kernel_spmd(
        nc, in_maps, core_ids=list(range(N_CORES)),
        trace=trace, **(trace_kwargs or {}))
    out = np.concatenate([res.results[c]["out"] for c in range(N_CORES)], axis=0)
    return out, res


def kernel(input, dir, prob):
    input = np.asarray(input, dtype=np.float32)
    dir = np.asarray(dir, dtype=np.int32)
    prob = np.asarray(prob, dtype=np.int32)
    out, _ = run(input, dir, prob, trace=False)
    return out



# revision 18
# speedup vs baseline: 1.1057x; 1.1024x over previous
"""Trainium2 Bass kernel for nn_Brown: masked directional pixel scatter + 3x3 avg.

Semantics (per image, last two dims H, W):
  pos  = prob <= 20
  avg  = 3x3 reflect-padded box mean of input
  for d in 0..7 sequentially (OFFSETS below):
      m = (dir == d) & pos
      if d == 4: x[m] = avg[m]
      else:      x[q + OFF] = input[q] for masked q (target in range),
                 then x[q] = avg[q] (for q with valid target)

Kernel formulation (validated vs reference in numpy):
  key1 = (dir+1) * (+1 if pos else -1)            in {-8..-1, 1..8}  (bf16)
  Z    = relu(key1), zeroed where the self-target is out of range    (bf16)
  out  = input copy; self-write first: out = avg where Z != 0
  for d ascending (d != 4), target rectangle p = q + OFF in range:
      u_d = relu((d+1) - Z)                  (ACT; !=0 iff Z < d+1)
      m_d = (key1[q] == d+1) * u_d           (STT; !=0 iff neighbor-write wins)
      out[p] = input[q] where m_d != 0       (copy_predicated)
  Ascending overwrite order resolves neighbor-vs-neighbor priority; the
  Z-blocking term resolves self-vs-neighbor priority exactly.

Sharding: fully data-parallel on batch, 4 batches per core x 8 cores.
"""

import numpy as np

import concourse.bass as bass
import concourse.bacc as bacc
import concourse.mybir as mybir
from concourse import tile
from concourse import bass_utils

AL = mybir.AluOpType
AF = mybir.ActivationFunctionType
DT = mybir.dt

B, C, H, W = 32, 64, 128, 128
N_CORES = 8
PB = B // N_CORES          # batches per core
NIMG = PB * C              # images per core
NGRP = NIMG // 128         # partition groups of 128 images
R = 16                     # strip rows
NSTRIP = H // R
P_THRESH = 20

# direction -> (di, dj); d=4 is the self (avg-only) case
OFFSETS = {0: (-1, -1), 1: (-1, 0), 2: (-1, 1), 3: (0, -1),
           5: (0, 1), 6: (1, -1), 7: (1, 0)}


def _register_consts(nc, values, dtype=DT.float32):
    for v in values:
        if (dtype, v) in nc.const_aps.aps:
            continue
        t = nc.alloc_sbuf_tensor(f"const-{dtype.name}-{v}", [128, 1], dtype)
        nc.gpsimd.memset(t.ap(), v)
        nc.const_aps.aps[(dtype, v)] = t.ap()
    nc.all_engine_barrier()


def build_brown(nc: bass.Bass, repeat: int = 1, variant: str = 'full'):
    """Emit the full per-core kernel into nc (one SPMD program)."""
    f32, bf16, i32 = DT.float32, DT.bfloat16, DT.int32
    _register_consts(nc, [20.5, 1.0 / 9.0] + [float(d + 1) for d in OFFSETS])
    inp = nc.dram_tensor("input", [PB, C, H, W], f32, kind="ExternalInput") \
            .ap().rearrange("b c h w -> (b c) h w")
    drm = nc.dram_tensor("dir", [PB, C, H, W], i32, kind="ExternalInput") \
            .ap().rearrange("b c h w -> (b c) h w")
    prm = nc.dram_tensor("prob", [PB, C, H, W], i32, kind="ExternalInput") \
            .ap().rearrange("b c h w -> (b c) h w")
    orm = nc.dram_tensor("out", [PB, C, H, W], f32, kind="ExternalOutput") \
            .ap().rearrange("b c h w -> (b c) h w")

    with tile.TileContext(nc) as tc:
        with tc.tile_pool(name="io", bufs=2) as pio, \
             tc.tile_pool(name="mk", bufs=2) as pmk:
            if repeat == 0:     # overhead-measurement variant: minimal work
                z = pio.tile([128, W], f32, tag="x")
                nc.sync.dma_start(z[:], inp[0:128, 0, :])
                nc.sync.dma_start(orm[0:128, 0, :], z[:])
            for _ in range(repeat):
                for g in range(NGRP):
                    for s in range(NSTRIP):
                        _strip(nc, pio, pmk, inp, drm, prm, orm, g, s, variant)
    return nc


def _strip(nc, pio, pmk, inp, drm, prm, orm, g, s, variant='full'):
    """One [128 images x R rows] strip. Tile row h <-> image row r0-1+h."""
    f32, bf16, i32 = DT.float32, DT.bfloat16, DT.int32
    r0 = s * R
    isl = slice(g * 128, (g + 1) * 128)
    first, last = (s == 0), (s == NSTRIP - 1)

    x = pio.tile([128, R + 2, W], f32, tag="x", bufs=3)
    dr = pio.tile([128, R + 2, W], i32, tag="dr", bufs=3)
    pr = pio.tile([128, R + 2, W], i32, tag="pr", bufs=3)

    # ---- loads (halo rows: reflect for input; dir/prob halo handled via key memset)
    if first:
        nc.sync.dma_start(x[:, 1:R + 2, :], inp[isl, 0:R + 1, :])
        nc.sync.dma_start(x[:, 0:1, :], inp[isl, 1:2, :])          # reflect row -1 -> 1
        nc.sync.dma_start(dr[:, 1:R + 2, :], drm[isl, 0:R + 1, :])
        nc.sync.dma_start(pr[:, 1:R + 2, :], prm[isl, 0:R + 1, :])
    elif last:
        nc.sync.dma_start(x[:, 0:R + 1, :], inp[isl, r0 - 1:H, :])
        nc.sync.dma_start(x[:, R + 1:R + 2, :], inp[isl, H - 2:H - 1, :])  # reflect
        nc.sync.dma_start(dr[:, 0:R + 1, :], drm[isl, r0 - 1:H, :])
        nc.sync.dma_start(pr[:, 0:R + 1, :], prm[isl, r0 - 1:H, :])
    else:
        nc.sync.dma_start(x[:], inp[isl, r0 - 1:r0 + R + 1, :])
        nc.sync.dma_start(dr[:], drm[isl, r0 - 1:r0 + R + 1, :])
        nc.sync.dma_start(pr[:], prm[isl, r0 - 1:r0 + R + 1, :])

    # ---- key1 = (dir+1) * sign(20.5 - prob)   (bf16, R+2 rows)
    v0, v1 = (1 if first else 0), (R + 1 if last else R + 2)   # loaded row range
    vs = slice(v0, v1)
    ds1 = pmk.tile([128, R + 2, W], bf16, tag="ds1")
    ps = pmk.tile([128, R + 2, W], bf16, tag="ps")
    nc.scalar.activation(ds1[:, vs, :], dr[:, vs, :], AF.Identity, bias=1.0, scale=1.0)
    nc.scalar.activation(ps[:, vs, :], pr[:, vs, :], AF.Sign, bias=20.5, scale=-1.0)
    key = pmk.tile([128, R + 2, W], bf16, tag="key")
    nc.vector.tensor_mul(key[:, vs, :], ds1[:, vs, :], ps[:, vs, :])
    # out-of-image halo: -9 matches neither any gate value d+1 nor u'==0
    if first:
        nc.vector.memset(key[:, 0:1, :], -9.0)
    if last:
        nc.vector.memset(key[:, R + 1:R + 2, :], -9.0)

    # ---- Z = relu(key) with out-of-range self-targets zeroed
    # (int16: copy_predicated masks must be integer dtype per BIR verifier)
    Z = pmk.tile([128, R, W], DT.int16, tag="Z")
    nc.scalar.activation(Z[:], key[:, 1:R + 1, :], AF.Relu)   # relu on ACT engine
    if first:   # image row 0: self-dirs {0,1,2} (keys 1,2,3) invalid -> keep Z>=4
        nc.vector.scalar_tensor_tensor(Z[:, 0:1, :], Z[:, 0:1, :], 4.0,
                                       Z[:, 0:1, :], AL.is_ge, AL.mult)
    if last:    # image row 127: self-dirs {6,7} (keys 7,8) invalid -> keep Z<=6
        nc.vector.scalar_tensor_tensor(Z[:, R - 1:R, :], Z[:, R - 1:R, :], 6.0,
                                       Z[:, R - 1:R, :], AL.is_le, AL.mult)
    # col 0: self-dirs {0,3,6} (keys 1,4,7) invalid
    for k in (1.0, 4.0, 7.0):
        nc.vector.scalar_tensor_tensor(Z[:, :, 0:1], Z[:, :, 0:1], k,
                                       Z[:, :, 0:1], AL.not_equal, AL.mult)
    # col 127: self-dirs {2,5} (keys 3,6) invalid
    for k in (3.0, 6.0):
        nc.vector.scalar_tensor_tensor(Z[:, :, W - 1:W], Z[:, :, W - 1:W], k,
                                       Z[:, :, W - 1:W], AL.not_equal, AL.mult)

    # ---- avg = 3x3 reflect box mean, summed in bf16 (DVE 2x mode), /9 folded
    # into the f32->bf16 convert; final upconvert to f32 on the scalar engine.
    do_avg = variant not in ("noavg", "min")
    do_scan = variant not in ("noscan", "min")
    xb9 = pmk.tile([128, R + 2, W], bf16, tag="xb9")
    t = pmk.tile([128, R + 2, W], bf16, tag="t")
    avgb = pmk.tile([128, R, W], bf16, tag="avgb")
    avg = pio.tile([128, R, W], f32, tag="avg", bufs=2)
    if do_avg:
        nc.scalar.activation(xb9[:], x[:], AF.Identity, scale=1.0 / 9.0)
        nc.vector.tensor_add(t[:, :, 1:W - 1], xb9[:, :, 0:W - 2], xb9[:, :, 2:W])
        # reflect columns on the ACT engine (vector is the bottleneck)
        nc.scalar.activation(t[:, :, 0:1], xb9[:, :, 1:2], AF.Identity, scale=2.0)
        nc.scalar.activation(t[:, :, W - 1:W], xb9[:, :, W - 2:W - 1],
                             AF.Identity, scale=2.0)
        nc.vector.tensor_add(t[:], t[:], xb9[:])
        nc.vector.tensor_add(avgb[:], t[:, 0:R, :], t[:, 2:R + 2, :])
        nc.vector.tensor_add(avgb[:], avgb[:], t[:, 1:R + 1, :])
        nc.scalar.activation(avg[:], avgb[:], AF.Identity)

    # ---- out = input; self-write first
    outt = pio.tile([128, R, W], f32, tag="outt", bufs=3)
    nc.sync.dma_start(outt[:], x[:, 1:R + 1, :])
    if do_avg:
        nc.vector.copy_predicated(outt[:], Z[:], avg[:])

    # ---- neighbor scan, ascending d
    for d, (di, dj) in (OFFSETS.items() if do_scan else []):
        c0, c1 = max(dj, 0), W + min(dj, 0)      # target col range
        # u = (Z < d+1) * (d+1): TS dual-op in 4x mode; then the mask is a
        # plain TT is_equal in 2x mode (key is never 0, so key==u <=> both
        # conditions) — cheaper than the no-fast-mode STT form.
        u = pmk.tile([128, R, W], bf16, tag="u")
        nc.vector.tensor_scalar(u[:], Z[:], float(d + 1), float(d + 1),
                                AL.is_lt, AL.mult)
        m = pmk.tile([128, R, W], DT.int16, tag="m", bufs=1)
        nc.vector.tensor_tensor(
            m[:, :, c0:c1],
            key[:, 1 - di:1 - di + R, c0 - dj:c1 - dj],
            u[:, :, c0:c1], AL.is_equal)
        nc.vector.copy_predicated(
            outt[:, :, c0:c1], m[:, :, c0:c1],
            x[:, 1 - di:1 - di + R, c0 - dj:c1 - dj])

    nc.sync.dma_start(orm[isl, r0:r0 + R, :], outt[:])


_CACHE = {}


def _get_nc(repeat: int = 1, variant: str = "full"):
    k = ("nc", repeat, variant)
    if k not in _CACHE:
        nc = bacc.Bacc("TRN2", target_bir_lowering=False, debug=False)
        build_brown(nc, repeat=repeat, variant=variant)
        nc.compile()
        _CACHE[k] = nc
    return _CACHE[k]


def run(input, dir, prob, trace=False, trace_kwargs=None, repeat=1):
    """Shard over batch, run on 8 cores, gather. Returns (out, BassKernelResults)."""
    nc = _get_nc(repeat)
    in_maps = []
    for c in range(N_CORES):
        bs = slice(c * PB, (c + 1) * PB)
        in_maps.append({
            "input": np.ascontiguousarray(input[bs]),
            "dir": np.ascontiguousarray(dir[bs]),
            "prob": np.ascontiguousarray(prob[bs]),
        })
    res = bass_utils.run_bass_kernel_spmd(
        nc, in_maps, core_ids=list(range(N_CORES)),
        trace=trace, **(trace_kwargs or {}))
    out = np.concatenate([res.results[c]["out"] for c in range(N_CORES)], axis=0)
    return out, res


def kernel(input, dir, prob):
    input = np.asarray(input, dtype=np.float32)
    dir = np.asarray(dir, dtype=np.int32)
    prob = np.asarray(prob, dtype=np.int32)
    out, _ = run(input, dir, prob, trace=False)
    return out



# revision 20
# speedup vs baseline: 1.1644x; 1.0531x over previous
"""Trainium2 Bass kernel for nn_Brown: masked directional pixel scatter + 3x3 avg.

Semantics (per image, last two dims H, W):
  pos  = prob <= 20
  avg  = 3x3 reflect-padded box mean of input
  for d in 0..7 sequentially (OFFSETS below):
      m = (dir == d) & pos
      if d == 4: x[m] = avg[m]
      else:      x[q + OFF] = input[q] for masked q (target in range),
                 then x[q] = avg[q] (for q with valid target)

Kernel formulation (validated vs reference in numpy):
  key1 = (dir+1) * (+1 if pos else -1)            in {-8..-1, 1..8}  (bf16)
  Z    = relu(key1), zeroed where the self-target is out of range    (bf16)
  out  = input copy; self-write first: out = avg where Z != 0
  for d ascending (d != 4), target rectangle p = q + OFF in range:
      u_d = relu((d+1) - Z)                  (ACT; !=0 iff Z < d+1)
      m_d = (key1[q] == d+1) * u_d           (STT; !=0 iff neighbor-write wins)
      out[p] = input[q] where m_d != 0       (copy_predicated)
  Ascending overwrite order resolves neighbor-vs-neighbor priority; the
  Z-blocking term resolves self-vs-neighbor priority exactly.

Sharding: fully data-parallel on batch, 4 batches per core x 8 cores.
"""

import numpy as np

import concourse.bass as bass
import concourse.bacc as bacc
import concourse.mybir as mybir
from concourse import tile
from concourse import bass_utils

AL = mybir.AluOpType
AF = mybir.ActivationFunctionType
DT = mybir.dt

B, C, H, W = 32, 64, 128, 128
N_CORES = 8
PB = B // N_CORES          # batches per core
NIMG = PB * C              # images per core
NGRP = NIMG // 128         # partition groups of 128 images
R = 16                     # strip rows
NSTRIP = H // R
P_THRESH = 20

# direction -> (di, dj); d=4 is the self (avg-only) case
OFFSETS = {0: (-1, -1), 1: (-1, 0), 2: (-1, 1), 3: (0, -1),
           5: (0, 1), 6: (1, -1), 7: (1, 0)}


def _register_consts(nc, values, dtype=DT.float32):
    for v in values:
        if (dtype, v) in nc.const_aps.aps:
            continue
        t = nc.alloc_sbuf_tensor(f"const-{dtype.name}-{v}", [128, 1], dtype)
        nc.gpsimd.memset(t.ap(), v)
        nc.const_aps.aps[(dtype, v)] = t.ap()
    nc.all_engine_barrier()


def build_brown(nc: bass.Bass, repeat: int = 1, variant: str = 'full'):
    """Emit the full per-core kernel into nc (one SPMD program)."""
    f32, bf16, i32 = DT.float32, DT.bfloat16, DT.int32
    _register_consts(nc, [20.5, 1.0 / 9.0] + [float(d + 1) for d in OFFSETS]
                     + [float(d) + 0.5 for d in OFFSETS])
    inp = nc.dram_tensor("input", [PB, C, H, W], f32, kind="ExternalInput") \
            .ap().rearrange("b c h w -> (b c) h w")
    drm = nc.dram_tensor("dir", [PB, C, H, W], i32, kind="ExternalInput") \
            .ap().rearrange("b c h w -> (b c) h w")
    prm = nc.dram_tensor("prob", [PB, C, H, W], i32, kind="ExternalInput") \
            .ap().rearrange("b c h w -> (b c) h w")
    orm = nc.dram_tensor("out", [PB, C, H, W], f32, kind="ExternalOutput") \
            .ap().rearrange("b c h w -> (b c) h w")

    with tile.TileContext(nc) as tc:
        with tc.tile_pool(name="io", bufs=2) as pio, \
             tc.tile_pool(name="mk", bufs=2) as pmk:
            if repeat == 0:     # overhead-measurement variant: minimal work
                z = pio.tile([128, W], f32, tag="x")
                nc.sync.dma_start(z[:], inp[0:128, 0, :])
                nc.sync.dma_start(orm[0:128, 0, :], z[:])
            for _ in range(repeat):
                for g in range(NGRP):
                    for s in range(NSTRIP):
                        _strip(nc, pio, pmk, inp, drm, prm, orm, g, s, variant)
    return nc


def _strip(nc, pio, pmk, inp, drm, prm, orm, g, s, variant='full'):
    """One [128 images x R rows] strip. Tile row h <-> image row r0-1+h."""
    f32, bf16, i32 = DT.float32, DT.bfloat16, DT.int32
    r0 = s * R
    isl = slice(g * 128, (g + 1) * 128)
    first, last = (s == 0), (s == NSTRIP - 1)

    x = pio.tile([128, R + 2, W], f32, tag="x", bufs=3)
    dr = pio.tile([128, R + 2, W], i32, tag="dr", bufs=3)
    pr = pio.tile([128, R + 2, W], i32, tag="pr", bufs=3)

    # ---- loads (halo rows: reflect for input; dir/prob halo handled via key memset)
    if first:
        nc.sync.dma_start(x[:, 1:R + 2, :], inp[isl, 0:R + 1, :])
        nc.sync.dma_start(x[:, 0:1, :], inp[isl, 1:2, :])          # reflect row -1 -> 1
        nc.sync.dma_start(dr[:, 1:R + 2, :], drm[isl, 0:R + 1, :])
        nc.sync.dma_start(pr[:, 1:R + 2, :], prm[isl, 0:R + 1, :])
    elif last:
        nc.sync.dma_start(x[:, 0:R + 1, :], inp[isl, r0 - 1:H, :])
        nc.sync.dma_start(x[:, R + 1:R + 2, :], inp[isl, H - 2:H - 1, :])  # reflect
        nc.sync.dma_start(dr[:, 0:R + 1, :], drm[isl, r0 - 1:H, :])
        nc.sync.dma_start(pr[:, 0:R + 1, :], prm[isl, r0 - 1:H, :])
    else:
        nc.sync.dma_start(x[:], inp[isl, r0 - 1:r0 + R + 1, :])
        nc.sync.dma_start(dr[:], drm[isl, r0 - 1:r0 + R + 1, :])
        nc.sync.dma_start(pr[:], prm[isl, r0 - 1:r0 + R + 1, :])

    # ---- key1 = (dir+1) * sign(20.5 - prob)   (bf16, R+2 rows)
    v0, v1 = (1 if first else 0), (R + 1 if last else R + 2)   # loaded row range
    vs = slice(v0, v1)
    ds1 = pmk.tile([128, R + 2, W], bf16, tag="ds1")
    ps = pmk.tile([128, R + 2, W], bf16, tag="ps")
    nc.scalar.activation(ds1[:, vs, :], dr[:, vs, :], AF.Identity, bias=1.0, scale=1.0)
    nc.scalar.activation(ps[:, vs, :], pr[:, vs, :], AF.Sign, bias=20.5, scale=-1.0)
    key = pmk.tile([128, R + 2, W], bf16, tag="key")
    nc.vector.tensor_mul(key[:, vs, :], ds1[:, vs, :], ps[:, vs, :])
    # out-of-image halo: -9 matches neither any gate value d+1 nor u'==0
    if first:
        nc.vector.memset(key[:, 0:1, :], -9.0)
    if last:
        nc.vector.memset(key[:, R + 1:R + 2, :], -9.0)

    # ---- Z = relu(key) with out-of-range self-targets zeroed
    # (int16: copy_predicated masks must be integer dtype per BIR verifier)
    Z = pmk.tile([128, R, W], DT.int16, tag="Z")
    nc.scalar.activation(Z[:], key[:, 1:R + 1, :], AF.Relu)   # relu on ACT engine
    if first:   # image row 0: self-dirs {0,1,2} (keys 1,2,3) invalid -> keep Z>=4
        nc.vector.scalar_tensor_tensor(Z[:, 0:1, :], Z[:, 0:1, :], 4.0,
                                       Z[:, 0:1, :], AL.is_ge, AL.mult)
    if last:    # image row 127: self-dirs {6,7} (keys 7,8) invalid -> keep Z<=6
        nc.vector.scalar_tensor_tensor(Z[:, R - 1:R, :], Z[:, R - 1:R, :], 6.0,
                                       Z[:, R - 1:R, :], AL.is_le, AL.mult)
    # col 0: self-dirs {0,3,6} (keys 1,4,7) invalid
    for k in (1.0, 4.0, 7.0):
        nc.vector.scalar_tensor_tensor(Z[:, :, 0:1], Z[:, :, 0:1], k,
                                       Z[:, :, 0:1], AL.not_equal, AL.mult)
    # col 127: self-dirs {2,5} (keys 3,6) invalid
    for k in (3.0, 6.0):
        nc.vector.scalar_tensor_tensor(Z[:, :, W - 1:W], Z[:, :, W - 1:W], k,
                                       Z[:, :, W - 1:W], AL.not_equal, AL.mult)

    # ---- avg = 3x3 reflect box mean, summed in bf16 (DVE 2x mode), /9 folded
    # into the f32->bf16 convert; final upconvert to f32 on the scalar engine.
    do_avg = variant not in ("noavg", "min")
    do_scan = variant not in ("noscan", "min")
    xb9 = pmk.tile([128, R + 2, W], bf16, tag="xb9")
    t = pmk.tile([128, R + 2, W], bf16, tag="t")
    avgb = pmk.tile([128, R, W], bf16, tag="avgb")
    avg = pio.tile([128, R, W], f32, tag="avg", bufs=2)
    if do_avg:
        nc.scalar.activation(xb9[:], x[:], AF.Identity, scale=1.0 / 9.0)
        nc.vector.tensor_add(t[:, :, 1:W - 1], xb9[:, :, 0:W - 2], xb9[:, :, 2:W])
        # reflect columns on the ACT engine (vector is the bottleneck)
        nc.scalar.activation(t[:, :, 0:1], xb9[:, :, 1:2], AF.Identity, scale=2.0)
        nc.scalar.activation(t[:, :, W - 1:W], xb9[:, :, W - 2:W - 1],
                             AF.Identity, scale=2.0)
        nc.vector.tensor_add(t[:], t[:], xb9[:])
        nc.vector.tensor_add(avgb[:], t[:, 0:R, :], t[:, 2:R + 2, :])
        nc.vector.tensor_add(avgb[:], avgb[:], t[:, 1:R + 1, :])
        nc.scalar.activation(avg[:], avgb[:], AF.Identity)

    # ---- out = input; self-write first
    outt = pio.tile([128, R, W], f32, tag="outt", bufs=3)
    nc.sync.dma_start(outt[:], x[:, 1:R + 1, :])
    if do_avg:
        nc.vector.copy_predicated(outt[:], Z[:], avg[:])

    # ---- neighbor scan, ascending d
    for d, (di, dj) in (OFFSETS.items() if do_scan else []):
        c0, c1 = max(dj, 0), W + min(dj, 0)      # target col range
        # u = (Z < d+1) * (d+1); then the mask is a plain TT is_equal in 2x
        # mode (key is never 0, so key==u <=> both conditions). For most
        # directions u is built on the idle ACT engine via a 2-ACT chain
        # (sign then scaled relu -> exactly {0, d+1}); the rest stay on the
        # vector engine as a 4x-mode dual-op TS to balance the two queues.
        u = pmk.tile([128, R, W], bf16, tag="u")
        if d in (0, 1, 2, 3, 5):
            us = pmk.tile([128, R, W], bf16, tag="us")
            nc.scalar.activation(us[:], Z[:], AF.Sign,
                                 bias=float(d) + 0.5, scale=-1.0)
            nc.scalar.activation(u[:], us[:], AF.Relu, scale=float(d + 1))
        else:
            nc.vector.tensor_scalar(u[:], Z[:], float(d + 1), float(d + 1),
                                    AL.is_lt, AL.mult)
        m = pmk.tile([128, R, W], DT.int16, tag="m", bufs=1)
        nc.vector.tensor_tensor(
            m[:, :, c0:c1],
            key[:, 1 - di:1 - di + R, c0 - dj:c1 - dj],
            u[:, :, c0:c1], AL.is_equal)
        nc.vector.copy_predicated(
            outt[:, :, c0:c1], m[:, :, c0:c1],
            x[:, 1 - di:1 - di + R, c0 - dj:c1 - dj])

    nc.sync.dma_start(orm[isl, r0:r0 + R, :], outt[:])


_CACHE = {}


def _get_nc(repeat: int = 1, variant: str = "full"):
    k = ("nc", repeat, variant)
    if k not in _CACHE:
        nc = bacc.Bacc("TRN2", target_bir_lowering=False, debug=False)
        build_brown(nc, repeat=repeat, variant=variant)
        nc.compile()
        _CACHE[k] = nc
    return _CACHE[k]


def run(input, dir, prob, trace=False, trace_kwargs=None, repeat=1):
    """Shard over batch, run on 8 cores, gather. Returns (out, BassKernelResults)."""
    nc = _get_nc(repeat)
    in_maps = []
    for c in range(N_CORES):
        bs = slice(c * PB, (c + 1) * PB)
        in_maps.append({
            "input": np.ascontiguousarray(input[bs]),
            "dir": np.ascontiguousarray(dir[bs]),
            "prob": np.ascontiguousarray(prob[bs]),
        })
    res = bass_utils.run_bass_kernel_spmd(
        nc, in_maps, core_ids=list(range(N_CORES)),
        trace=trace, **(trace_kwargs or {}))
    out = np.concatenate([res.results[c]["out"] for c in range(N_CORES)], axis=0)
    return out, res


def kernel(input, dir, prob):
    input = np.asarray(input, dtype=np.float32)
    dir = np.asarray(dir, dtype=np.int32)
    prob = np.asarray(prob, dtype=np.int32)
    out, _ = run(input, dir, prob, trace=False)
    return out



# revision 21
# speedup vs baseline: 1.1662x; 1.0016x over previous
"""Trainium2 Bass kernel for nn_Brown: masked directional pixel scatter + 3x3 avg.

Semantics (per image, last two dims H, W):
  pos  = prob <= 20
  avg  = 3x3 reflect-padded box mean of input
  for d in 0..7 sequentially (OFFSETS below):
      m = (dir == d) & pos
      if d == 4: x[m] = avg[m]
      else:      x[q + OFF] = input[q] for masked q (target in range),
                 then x[q] = avg[q] (for q with valid target)

Kernel formulation (validated vs reference in numpy):
  key1 = (dir+1) * (+1 if pos else -1)            in {-8..-1, 1..8}  (bf16)
  Z    = relu(key1), zeroed where the self-target is out of range    (bf16)
  out  = input copy; self-write first: out = avg where Z != 0
  for d ascending (d != 4), target rectangle p = q + OFF in range:
      u_d = relu((d+1) - Z)                  (ACT; !=0 iff Z < d+1)
      m_d = (key1[q] == d+1) * u_d           (STT; !=0 iff neighbor-write wins)
      out[p] = input[q] where m_d != 0       (copy_predicated)
  Ascending overwrite order resolves neighbor-vs-neighbor priority; the
  Z-blocking term resolves self-vs-neighbor priority exactly.

Sharding: fully data-parallel on batch, 4 batches per core x 8 cores.
"""

import numpy as np

import concourse.bass as bass
import concourse.bacc as bacc
import concourse.mybir as mybir
from concourse import tile
from concourse import bass_utils

AL = mybir.AluOpType
AF = mybir.ActivationFunctionType
DT = mybir.dt

B, C, H, W = 32, 64, 128, 128
N_CORES = 8
PB = B // N_CORES          # batches per core
NIMG = PB * C              # images per core
NGRP = NIMG // 128         # partition groups of 128 images
R = 16                     # strip rows
NSTRIP = H // R
P_THRESH = 20

# direction -> (di, dj); d=4 is the self (avg-only) case
OFFSETS = {0: (-1, -1), 1: (-1, 0), 2: (-1, 1), 3: (0, -1),
           5: (0, 1), 6: (1, -1), 7: (1, 0)}


def _register_consts(nc, values, dtype=DT.float32):
    for v in values:
        if (dtype, v) in nc.const_aps.aps:
            continue
        t = nc.alloc_sbuf_tensor(f"const-{dtype.name}-{v}", [128, 1], dtype)
        nc.gpsimd.memset(t.ap(), v)
        nc.const_aps.aps[(dtype, v)] = t.ap()
    nc.all_engine_barrier()


def build_brown(nc: bass.Bass, repeat: int = 1, variant: str = 'full'):
    """Emit the full per-core kernel into nc (one SPMD program)."""
    f32, bf16, i32 = DT.float32, DT.bfloat16, DT.int32
    _register_consts(nc, [20.5, 1.0 / 9.0] + [float(d + 1) for d in OFFSETS]
                     + [float(d) + 0.5 for d in OFFSETS])
    inp = nc.dram_tensor("input", [PB, C, H, W], f32, kind="ExternalInput") \
            .ap().rearrange("b c h w -> (b c) h w")
    drm = nc.dram_tensor("dir", [PB, C, H, W], i32, kind="ExternalInput") \
            .ap().rearrange("b c h w -> (b c) h w")
    prm = nc.dram_tensor("prob", [PB, C, H, W], i32, kind="ExternalInput") \
            .ap().rearrange("b c h w -> (b c) h w")
    orm = nc.dram_tensor("out", [PB, C, H, W], f32, kind="ExternalOutput") \
            .ap().rearrange("b c h w -> (b c) h w")

    with tile.TileContext(nc) as tc:
        with tc.tile_pool(name="io", bufs=2) as pio, \
             tc.tile_pool(name="mk", bufs=2) as pmk:
            if repeat == 0:     # overhead-measurement variant: minimal work
                z = pio.tile([128, W], f32, tag="x")
                nc.sync.dma_start(z[:], inp[0:128, 0, :])
                nc.sync.dma_start(orm[0:128, 0, :], z[:])
            for _ in range(repeat):
                for g in range(NGRP):
                    for s in range(NSTRIP):
                        _strip(nc, pio, pmk, inp, drm, prm, orm, g, s, variant)
    return nc


def _strip(nc, pio, pmk, inp, drm, prm, orm, g, s, variant='full'):
    """One [128 images x R rows] strip. Tile row h <-> image row r0-1+h."""
    f32, bf16, i32 = DT.float32, DT.bfloat16, DT.int32
    r0 = s * R
    isl = slice(g * 128, (g + 1) * 128)
    first, last = (s == 0), (s == NSTRIP - 1)

    x = pio.tile([128, R + 2, W], f32, tag="x", bufs=3)
    dr = pio.tile([128, R + 2, W], i32, tag="dr", bufs=3)
    pr = pio.tile([128, R + 2, W], i32, tag="pr", bufs=3)

    # ---- loads (halo rows: reflect for input; dir/prob halo handled via key memset)
    if first:
        nc.sync.dma_start(x[:, 1:R + 2, :], inp[isl, 0:R + 1, :])
        nc.sync.dma_start(x[:, 0:1, :], inp[isl, 1:2, :])          # reflect row -1 -> 1
        nc.sync.dma_start(dr[:, 1:R + 2, :], drm[isl, 0:R + 1, :])
        nc.sync.dma_start(pr[:, 1:R + 2, :], prm[isl, 0:R + 1, :])
    elif last:
        nc.sync.dma_start(x[:, 0:R + 1, :], inp[isl, r0 - 1:H, :])
        nc.sync.dma_start(x[:, R + 1:R + 2, :], inp[isl, H - 2:H - 1, :])  # reflect
        nc.sync.dma_start(dr[:, 0:R + 1, :], drm[isl, r0 - 1:H, :])
        nc.sync.dma_start(pr[:, 0:R + 1, :], prm[isl, r0 - 1:H, :])
    else:
        nc.sync.dma_start(x[:], inp[isl, r0 - 1:r0 + R + 1, :])
        nc.sync.dma_start(dr[:], drm[isl, r0 - 1:r0 + R + 1, :])
        nc.sync.dma_start(pr[:], prm[isl, r0 - 1:r0 + R + 1, :])

    # ---- key1 = (dir+1) * sign(20.5 - prob)   (bf16, R+2 rows)
    v0, v1 = (1 if first else 0), (R + 1 if last else R + 2)   # loaded row range
    vs = slice(v0, v1)
    ds1 = pmk.tile([128, R + 2, W], bf16, tag="ds1")
    ps = pmk.tile([128, R + 2, W], bf16, tag="ps")
    nc.scalar.activation(ds1[:, vs, :], dr[:, vs, :], AF.Identity, bias=1.0, scale=1.0)
    nc.scalar.activation(ps[:, vs, :], pr[:, vs, :], AF.Sign, bias=20.5, scale=-1.0)
    key = pmk.tile([128, R + 2, W], bf16, tag="key")
    nc.vector.tensor_mul(key[:, vs, :], ds1[:, vs, :], ps[:, vs, :])
    # out-of-image halo: -9 matches neither any gate value d+1 nor u'==0
    if first:
        nc.vector.memset(key[:, 0:1, :], -9.0)
    if last:
        nc.vector.memset(key[:, R + 1:R + 2, :], -9.0)

    # ---- Z = relu(key) with out-of-range self-targets zeroed
    # (int16: copy_predicated masks must be integer dtype per BIR verifier)
    Z = pmk.tile([128, R, W], DT.int16, tag="Z")
    nc.scalar.activation(Z[:], key[:, 1:R + 1, :], AF.Relu)   # relu on ACT engine
    if first:   # image row 0: self-dirs {0,1,2} (keys 1,2,3) invalid -> keep Z>=4
        nc.vector.scalar_tensor_tensor(Z[:, 0:1, :], Z[:, 0:1, :], 4.0,
                                       Z[:, 0:1, :], AL.is_ge, AL.mult)
    if last:    # image row 127: self-dirs {6,7} (keys 7,8) invalid -> keep Z<=6
        nc.vector.scalar_tensor_tensor(Z[:, R - 1:R, :], Z[:, R - 1:R, :], 6.0,
                                       Z[:, R - 1:R, :], AL.is_le, AL.mult)
    # col 0: self-dirs {0,3,6} (keys 1,4,7) invalid
    for k in (1.0, 4.0, 7.0):
        nc.vector.scalar_tensor_tensor(Z[:, :, 0:1], Z[:, :, 0:1], k,
                                       Z[:, :, 0:1], AL.not_equal, AL.mult)
    # col 127: self-dirs {2,5} (keys 3,6) invalid
    for k in (3.0, 6.0):
        nc.vector.scalar_tensor_tensor(Z[:, :, W - 1:W], Z[:, :, W - 1:W], k,
                                       Z[:, :, W - 1:W], AL.not_equal, AL.mult)

    # ---- avg = 3x3 reflect box mean, summed in bf16 (DVE 2x mode), /9 folded
    # into the f32->bf16 convert; final upconvert to f32 on the scalar engine.
    do_avg = variant not in ("noavg", "min")
    do_scan = variant not in ("noscan", "min")
    xb9 = pmk.tile([128, R + 2, W], bf16, tag="xb9")
    t = pmk.tile([128, R + 2, W], bf16, tag="t")
    avgb = pmk.tile([128, R, W], bf16, tag="avgb")
    avg = pio.tile([128, R, W], f32, tag="avg", bufs=2)
    if do_avg:
        nc.scalar.activation(xb9[:], x[:], AF.Identity, scale=1.0 / 9.0)
        nc.vector.tensor_add(t[:, :, 1:W - 1], xb9[:, :, 0:W - 2], xb9[:, :, 2:W])
        # reflect columns on the ACT engine (vector is the bottleneck)
        nc.scalar.activation(t[:, :, 0:1], xb9[:, :, 1:2], AF.Identity, scale=2.0)
        nc.scalar.activation(t[:, :, W - 1:W], xb9[:, :, W - 2:W - 1],
                             AF.Identity, scale=2.0)
        nc.vector.tensor_add(t[:], t[:], xb9[:])
        nc.vector.tensor_add(avgb[:], t[:, 0:R, :], t[:, 2:R + 2, :])
        nc.vector.tensor_add(avgb[:], avgb[:], t[:, 1:R + 1, :])
        nc.scalar.activation(avg[:], avgb[:], AF.Identity)

    # ---- out = input; self-write first
    outt = pio.tile([128, R, W], f32, tag="outt", bufs=3)
    nc.sync.dma_start(outt[:], x[:, 1:R + 1, :])
    if do_avg:
        nc.vector.copy_predicated(outt[:], Z[:], avg[:])

    # ---- neighbor scan, ascending d
    for d, (di, dj) in (OFFSETS.items() if do_scan else []):
        c0, c1 = max(dj, 0), W + min(dj, 0)      # target col range
        # u = (Z < d+1) * (d+1); then the mask is a plain TT is_equal in 2x
        # mode (key is never 0, so key==u <=> both conditions). For most
        # directions u is built on the idle ACT engine via a 2-ACT chain
        # (sign then scaled relu -> exactly {0, d+1}); the rest stay on the
        # vector engine as a 4x-mode dual-op TS to balance the two queues.
        u = pmk.tile([128, R, W], bf16, tag="u")
        if d in (0, 1, 2, 3, 5, 6):
            us = pmk.tile([128, R, W], bf16, tag="us")
            nc.scalar.activation(us[:], Z[:], AF.Sign,
                                 bias=float(d) + 0.5, scale=-1.0)
            nc.scalar.activation(u[:], us[:], AF.Relu, scale=float(d + 1))
        else:
            nc.vector.tensor_scalar(u[:], Z[:], float(d + 1), float(d + 1),
                                    AL.is_lt, AL.mult)
        m = pmk.tile([128, R, W], DT.int16, tag="m", bufs=1)
        nc.vector.tensor_tensor(
            m[:, :, c0:c1],
            key[:, 1 - di:1 - di + R, c0 - dj:c1 - dj],
            u[:, :, c0:c1], AL.is_equal)
        nc.vector.copy_predicated(
            outt[:, :, c0:c1], m[:, :, c0:c1],
            x[:, 1 - di:1 - di + R, c0 - dj:c1 - dj])

    nc.sync.dma_start(orm[isl, r0:r0 + R, :], outt[:])


_CACHE = {}


def _get_nc(repeat: int = 1, variant: str = "full"):
    k = ("nc", repeat, variant)
    if k not in _CACHE:
        nc = bacc.Bacc("TRN2", target_bir_lowering=False, debug=False)
        build_brown(nc, repeat=repeat, variant=variant)
        nc.compile()
        _CACHE[k] = nc
    return _CACHE[k]


def run(input, dir, prob, trace=False, trace_kwargs=None, repeat=1):
    """Shard over batch, run on 8 cores, gather. Returns (out, BassKernelResults)."""
    nc = _get_nc(repeat)
    in_maps = []
    for c in range(N_CORES):
        bs = slice(c * PB, (c + 1) * PB)
        in_maps.append({
            "input": np.ascontiguousarray(input[bs]),
            "dir": np.ascontiguousarray(dir[bs]),
            "prob": np.ascontiguousarray(prob[bs]),
        })
    res = bass_utils.run_bass_kernel_spmd(
        nc, in_maps, core_ids=list(range(N_CORES)),
        trace=trace, **(trace_kwargs or {}))
    out = np.concatenate([res.results[c]["out"] for c in range(N_CORES)], axis=0)
    return out, res


def kernel(input, dir, prob):
    input = np.asarray(input, dtype=np.float32)
    dir = np.asarray(dir, dtype=np.int32)
    prob = np.asarray(prob, dtype=np.int32)
    out, _ = run(input, dir, prob, trace=False)
    return out

